# revision 16
# baseline (speedup 1.0000x reference)
"""BoxConv2d Trainium2 kernel.

Reference computes, per (c, f) box and batch b:
    out[b, c*FN+f, i, j] = integral of x[b, c] over the continuous window
        rows [i + x_min, i + x_max + 1) x cols [j + y_min, j + y_max + 1),
    with window coordinates clipped to [0, H] x [0, W] (bilinear sampling of
    the integral image is exact for piecewise-constant images).

That is exactly a separable band matmul with clamped-ramp overlap weights:
    Wx[i, p] = clamp01(p + 1 - (i + x_min)) - clamp01(p + 1 - (i + x_max + 1))
    Wy[j, q] = clamp01(q + 1 - (j + y_min)) - clamp01(q + 1 - (j + y_max + 1))
    out[b, cf] = Wx @ x[b, c] @ Wy^T

The Wx/Wy matrices depend only on the tiny box parameters, so they are built
on the host and shipped to the device; the device kernel is pure TensorE
matmuls in fp16 (fp32 PSUM accumulation), which numpy-validates to ~3e-4
relative error against the fp32 reference.

Sharding: channels across the 8 cores (4 channels/core, all 4 batches), box
parameters replicated per-core as part of each core's W shard.

Step 1 (x side):  V^B[j, f*256+io] = sum_p x[p, j] * Wx[f][io, p]
    lhsT (stationary) = x chunk [p-chunk, j-half], rhs = WxT [p-chunk, 2f*io].
Step 2 (y side):  out[ih*128+io, jo] = sum_j V[j, ...] * Wy[f][jo, j]
    lhsT = V chunk [j-chunk, io-half], rhs = WyT [j-chunk, jo].

DMA strategy (one dma_start fans across all 16 SDMA engines; big transfers
are efficient, small ones are not):
  - x is host-packed per channel as [128, b*512 + pc*256 + j] so each
    channel is one contiguous 256 KB load; wx/wy are 512 KB per channel.
  - Everything is prefetched up front (total ~6 MB, SBUF-resident);
    the first-needed chunks (c0 x for b0, c0 wx, c0 wy f0/f1) ride the
    HWDGE queues (sync/scalar, ~0.6us latency) so the PE starts early;
    the rest streams on the gpsimd (SWDGE) queue, which is empty long
    before the kernel tail (no end-of-kernel SWDGE drain).
  - Output tiles are stored as single 512 KB transfers on sync; the last
    two tiles split across sync+scalar with per-f chunks to shorten the
    final drain.
"""

import numpy as np

B, C, FN, H, W = 4, 32, 4, 256, 256
N_CORES = 8
C_PER_CORE = C // N_CORES  # 4 channels per core

_PROGRAM_CACHE = {}

N_WARM = 26  # HAM warmup matmuls (~107ns each cold) bridging until data lands


def _build_program():
    """Build (once) the SPMD Bass program run identically on all 8 cores."""
    import concourse.bass as bass
    import concourse.tile as tile
    from concourse import bacc, mybir

    nc = bacc.Bacc("TRN2", target_bir_lowering=False, debug=False)
    f16 = mybir.dt.float16
    f32 = mybir.dt.float32

    # Per-core inputs, host-laid-out so every DMA is one contiguous 2D copy:
    # xall[c, p, b*512 + pc*256 + j] = x[b, c, pc*128 + p, j]
    # wxt[c, p, (fp*2+pc)*512 + fi*256 + io] = Wx[c, 2fp+fi][io, pc*128 + p]
    # wyt[c, j, (f*2+jc)*256 + jo]      = Wy[c, f][jo, jc*128 + j]
    xall = nc.dram_tensor("xall", [C_PER_CORE, 128, B * 512], f16,
                          kind="ExternalInput").ap()
    wxt = nc.dram_tensor("wxt", [C_PER_CORE, 128, 2048], f16,
                         kind="ExternalInput").ap()
    wyt = nc.dram_tensor("wyt", [C_PER_CORE, 128, 2048], f16,
                         kind="ExternalInput").ap()
    # out_dev[b, c, p, f*512 + a*256 + jo] = out[b, c*FN+f, a*128+p, jo]
    # (host transposes back; keeps store DMAs fully contiguous per partition)
    # fp16 output (|out| <~1e3, fp16 quantization ~5e-4 rel; host upcasts):
    # halves store traffic, and the kernel tail is store-drain bound.
    out = nc.dram_tensor("out", [B, C_PER_CORE, 128, 2048], f16,
                         kind="ExternalOutput").ap()

    with tile.TileContext(nc, pool_alloc_mode="queue") as tc:
        with (
            tc.tile_pool(name="wx", bufs=4) as wx_pool,
            tc.tile_pool(name="wy", bufs=4) as wy_pool,
            tc.tile_pool(name="xin", bufs=4) as x_pool,
            tc.tile_pool(name="warm", bufs=1) as warm_pool,
            tc.tile_pool(name="v", bufs=8) as v_pool,
            tc.tile_pool(name="osb", bufs=6) as o_pool,
            tc.tile_pool(name="psv", bufs=4, space=bass.MemorySpace.PSUM) as psv_pool,
            tc.tile_pool(name="pso", bufs=4, space=bass.MemorySpace.PSUM) as pso_pool,
        ):
            # Warm the PE clock gate (HAM) during the initial load
            # latency with dependency-free matmuls on scratch data.
            warm_sb = warm_pool.tile([128, 128], f16, tag="warm_sb",
                                     name="warm_sb")
            nc.vector.memset(warm_sb[:], 0.0)
            warm_ps = pso_pool.tile([128, 512], f32, tag="pso", name="pso")
            for _w in range(N_WARM):
                nc.tensor.matmul(warm_ps[:, :128], warm_sb[:], warm_sb[:],
                                 start=True, stop=True)

            # ---- prefetch all inputs up front ----
            xt_c = []
            wx_c = []
            wy_c = []
            for c in range(C_PER_CORE):
                xt_c.append(x_pool.tile([128, B * 512], f16, tag="x",
                                        name="x"))
                wx_c.append(wx_pool.tile([128, 2048], f16, tag="wx",
                                         name="wx"))
                wy_c.append(wy_pool.tile([128, 2048], f16, tag="wy",
                                         name="wy"))
            # Early DMA delivery is receipt/turnaround-bound, not
            # bandwidth-bound: a queue's FIFO delivers strictly in order,
            # and concurrent queues share bandwidth at packet granularity
            # (which destroys need-ordering).  So the c0-critical prefix
            # rides ONE queue, strictly need-ordered and fine-chunked so
            # stage-1/2 of the first tiles consume chunk-by-chunk as they
            # land; the PE warmup bridges until the first chunks arrive.
            # c1-c3 then stream as big 512 KB whole-channel transfers,
            # done by ~+35us (the SWDGE queue is empty long before the
            # tail -> no end-of-kernel drain).
            # SWDGE (gpsimd) completion sems have a ~2us straggler engine
            # (descriptor-ring port contention) -- HWDGE sems complete
            # within ~0.5us of the data.  So the c0-critical chain rides
            # the two HWDGE queues, split by consumption order; only the
            # slack-rich c2/c3 bulk uses SWDGE.
            nc.sync.dma_start(xt_c[0][:, :512], xall[0][:, :512])
            nc.sync.dma_start(wx_c[0][:, :1024], wxt[0][:, :1024])
            nc.sync.dma_start(xt_c[0][:, 512:1024], xall[0][:, 512:1024])
            nc.sync.dma_start(xt_c[0][:, 1024:], xall[0][:, 1024:])
            nc.scalar.dma_start(wx_c[0][:, 1024:], wxt[0][:, 1024:])
            nc.scalar.dma_start(wy_c[0][:, :1024], wyt[0][:, :1024])
            nc.scalar.dma_start(wy_c[0][:, 1024:], wyt[0][:, 1024:])
            nc.scalar.dma_start(wx_c[1][:], wxt[1])
            nc.scalar.dma_start(xt_c[1][:], xall[1])
            nc.scalar.dma_start(wy_c[1][:], wyt[1])
            for c in range(2, C_PER_CORE):
                nc.gpsimd.dma_start(wx_c[c][:], wxt[c])
                nc.gpsimd.dma_start(xt_c[c][:], xall[c])
                nc.gpsimd.dma_start(wy_c[c][:], wyt[c])

            for c in range(C_PER_CORE):
                wx_t = wx_c[c]
                wy_t = wy_c[c]
                for b in range(B):
                    xt = xt_c[c]
                    xb = b * 512

                    # Step 1: one PSUM bank per (jh, f-pair); per-half
                    # casts so step 2's f0/f1 can start after the fp0 copy.
                    vt = [v_pool.tile([128, 1024], f16, tag="v", name="v")
                          for _jh in range(2)]
                    for jh in range(2):
                        for fp in range(2):
                            psv = psv_pool.tile([128, 512], f32, tag="psv",
                                                name="psv")
                            for pc in range(2):
                                nc.tensor.matmul(
                                    psv[:],
                                    xt[:, xb + pc * 256 + jh * 128:
                                       xb + pc * 256 + jh * 128 + 128],
                                    wx_t[:, (fp * 2 + pc) * 512:
                                         (fp * 2 + pc) * 512 + 512],
                                    start=(pc == 0),
                                    stop=(pc == 1),
                                )
                            eng = (nc.vector.tensor_copy if (jh + fp) % 2 == 0
                                   else nc.scalar.copy)
                            eng(vt[jh][:, fp * 512:(fp + 1) * 512], psv[:])

                    # Step 2
                    osb = o_pool.tile([128, 2048], f16, tag="o", name="osb")
                    last = c == C_PER_CORE - 1 and b == B - 1
                    for f in range(FN):
                        pso = pso_pool.tile([128, 512], f32, tag="pso",
                                            name="pso")
                        for ih in range(2):
                            for jc in range(2):
                                nc.tensor.matmul(
                                    pso[:, ih * 256:(ih + 1) * 256],
                                    vt[jc][:, f * 256 + ih * 128:
                                           f * 256 + ih * 128 + 128],
                                    wy_t[:, (f * 2 + jc) * 256:
                                         (f * 2 + jc) * 256 + 256],
                                    start=(jc == 0),
                                    stop=(jc == 1),
                                )
                        dst = osb[:, f * 512:(f + 1) * 512]
                        if last and f == FN - 1:
                            # very last chunk: copy halves on both engines in
                            # parallel, then one paired [f2 f3] store on
                            # scalar (sync already carries [f0 f1]) -- two
                            # final store issues instead of four
                            nc.vector.tensor_copy(dst[:, :256], pso[:, :256])
                            nc.scalar.copy(dst[:, 256:], pso[:, 256:])
                            nc.scalar.dma_start(out[b, c][:, 1024:],
                                                osb[:, 1024:])
                        else:
                            eng = (nc.vector.tensor_copy if f % 2 == 0
                                   else nc.scalar.copy)
                            eng(dst[:], pso[:])
                            if last and f == 1:
                                # first half of the final tile stores as
                                # soon as f0/f1 are copied
                                nc.sync.dma_start(out[b, c][:, :1024],
                                                  osb[:, :1024])
                    if not last:
                        if c == C_PER_CORE - 1 and b == B - 2:
                            # split across both HWDGE queues near the end
                            nc.sync.dma_start(out[b, c][:, :1024],
                                              osb[:, :1024])
                            nc.scalar.dma_start(out[b, c][:, 1024:],
                                                osb[:, 1024:])
                        else:
                            # single contiguous 512 KB store (best DMA
                            # efficiency); alternate queues in c2 so the two
                            # HWDGE queues share the late store backlog, but
                            # keep scalar's queue clear during c3 (it is
                            # needed for tail copies + the final store)
                            eng = (nc.scalar if c == 2 and b % 2 == 1
                                   else nc.sync)
                            eng.dma_start(out[b, c], osb[:])

    nc.compile()
    return nc


def _get_program():
    if "nc" not in _PROGRAM_CACHE:
        _PROGRAM_CACHE["nc"] = _build_program()
    return _PROGRAM_CACHE["nc"]


def _band(mn, mx, dim):
    """Overlap weights W[i, p] of clipped window [i+mn, i+mx+1) with cell
    [p, p+1), built in fp64."""
    i = np.arange(dim, dtype=np.float64)[:, None]
    p = np.arange(dim, dtype=np.float64)[None, :]
    lo = i + float(mn)
    hi = i + float(mx) + 1.0
    return np.clip(p + 1.0 - lo, 0.0, 1.0) - np.clip(p + 1.0 - hi, 0.0, 1.0)


def _prepare_in_maps(input, x_min, x_max, y_min, y_max):
    # xall[c, p, b*512 + pc*256 + j] = x[b, c, pc*128 + p, j]
    # (b-major free dim per channel so each channel is one contiguous load)
    x16_full = np.ascontiguousarray(
        input.astype(np.float16).reshape(B, C, 2, 128, 256)
        .transpose(1, 3, 0, 2, 4).reshape(C, 128, B * 512))

    in_maps = []
    for core in range(N_CORES):
        c0 = core * C_PER_CORE
        wxt = np.empty((C_PER_CORE, 128, 2048), dtype=np.float16)
        wyt = np.empty((C_PER_CORE, 128, 2048), dtype=np.float16)
        for cl in range(C_PER_CORE):
            c = c0 + cl
            for f in range(FN):
                WxT = _band(x_min[c, f], x_max[c, f], H).T.astype(np.float16)
                WyT = _band(y_min[c, f], y_max[c, f], W).T.astype(np.float16)
                fp, fi = f // 2, f % 2
                for pc in range(2):
                    base = (fp * 2 + pc) * 512 + fi * 256
                    wxt[cl, :, base:base + 256] = WxT[pc * 128:(pc + 1) * 128]
                for jc in range(2):
                    base = (f * 2 + jc) * 256
                    wyt[cl, :, base:base + 256] = WyT[jc * 128:(jc + 1) * 128]
        in_maps.append({
            "xall": np.ascontiguousarray(x16_full[c0:c0 + C_PER_CORE]),
            "wxt": wxt,
            "wyt": wyt,
        })
    return in_maps


def run(input, x_min, x_max, y_min, y_max, trace=False):
    """Run the SPMD kernel; returns (full_output, BassKernelResults)."""
    from concourse.bass_utils import run_bass_kernel_spmd

    nc = _get_program()
    in_maps = _prepare_in_maps(
        np.asarray(input, dtype=np.float32),
        np.asarray(x_min, dtype=np.float64),
        np.asarray(x_max, dtype=np.float64),
        np.asarray(y_min, dtype=np.float64),
        np.asarray(y_max, dtype=np.float64),
    )
    res = run_bass_kernel_spmd(nc, in_maps, list(range(N_CORES)), trace=trace)
    # out_dev[b, c, p, f*512 + a*256 + jo] -> out[b, c*FN+f, a*128+p, jo]
    parts = []
    for i in range(N_CORES):
        o = res.results[i]["out"].astype(np.float32).reshape(
            B, C_PER_CORE, 128, FN, 2, 256)
        parts.append(o.transpose(0, 1, 3, 4, 2, 5).reshape(
            B, C_PER_CORE * FN, 256, 256))
    full = np.ascontiguousarray(np.concatenate(parts, axis=1))
    return full, res


def kernel(input, x_min, x_max, y_min, y_max):
    full, _ = run(input, x_min, x_max, y_min, y_max)
    return full


# revision 18
# speedup vs baseline: 1.0582x; 1.0582x over previous
"""BoxConv2d Trainium2 kernel.

Reference computes, per (c, f) box and batch b:
    out[b, c*FN+f, i, j] = integral of x[b, c] over the continuous window
        rows [i + x_min, i + x_max + 1) x cols [j + y_min, j + y_max + 1),
    with window coordinates clipped to [0, H] x [0, W] (bilinear sampling of
    the integral image is exact for piecewise-constant images).

That is exactly a separable band matmul with clamped-ramp overlap weights:
    Wx[i, p] = clamp01(p + 1 - (i + x_min)) - clamp01(p + 1 - (i + x_max + 1))
    Wy[j, q] = clamp01(q + 1 - (j + y_min)) - clamp01(q + 1 - (j + y_max + 1))
    out[b, cf] = Wx @ x[b, c] @ Wy^T

The Wx/Wy matrices depend only on the tiny box parameters, so they are built
on the host and shipped to the device; the device kernel is pure TensorE
matmuls in fp16 (fp32 PSUM accumulation), which numpy-validates to ~3e-4
relative error against the fp32 reference.

Sharding: channels across the 8 cores (4 channels/core, all 4 batches), box
parameters replicated per-core as part of each core's W shard.

Step 1 (x side):  V^B[j, f*256+io] = sum_p x[p, j] * Wx[f][io, p]
    lhsT (stationary) = x chunk [p-chunk, j-half], rhs = WxT [p-chunk, 2f*io].
Step 2 (y side):  out[ih*128+io, jo] = sum_j V[j, ...] * Wy[f][jo, j]
    lhsT = V chunk [j-chunk, io-half], rhs = WyT [j-chunk, jo].

DMA strategy (one dma_start fans across all 16 SDMA engines; big transfers
are efficient, small ones are not):
  - x is host-packed per channel as [128, b*512 + pc*256 + j] so each
    channel is one contiguous 256 KB load; wx/wy are 512 KB per channel.
  - Everything is prefetched up front (total ~6 MB, SBUF-resident);
    the first-needed chunks (c0 x for b0, c0 wx, c0 wy f0/f1) ride the
    HWDGE queues (sync/scalar, ~0.6us latency) so the PE starts early;
    the rest streams on the gpsimd (SWDGE) queue, which is empty long
    before the kernel tail (no end-of-kernel SWDGE drain).
  - Output tiles are stored as single 512 KB transfers on sync; the last
    two tiles split across sync+scalar with per-f chunks to shorten the
    final drain.
"""

import numpy as np

B, C, FN, H, W = 4, 32, 4, 256, 256
N_CORES = 8
C_PER_CORE = C // N_CORES  # 4 channels per core

_PROGRAM_CACHE = {}

N_WARM = 32  # HAM warmup matmuls (~107ns each cold) bridging until data lands


def _build_program():
    """Build (once) the SPMD Bass program run identically on all 8 cores."""
    import concourse.bass as bass
    import concourse.tile as tile
    from concourse import bacc, mybir

    nc = bacc.Bacc("TRN2", target_bir_lowering=False, debug=False)
    f16 = mybir.dt.float16
    f32 = mybir.dt.float32

    # Per-core inputs, host-laid-out so every DMA is one contiguous 2D copy:
    # xall[c, p, b*512 + pc*256 + j] = x[b, c, pc*128 + p, j]
    # wxt[c, p, (fp*2+pc)*512 + fi*256 + io] = Wx[c, 2fp+fi][io, pc*128 + p]
    # wyt[c, j, (f*2+jc)*256 + jo]      = Wy[c, f][jo, jc*128 + j]
    xall = nc.dram_tensor("xall", [C_PER_CORE, 128, B * 512], f16,
                          kind="ExternalInput").ap()
    wxt = nc.dram_tensor("wxt", [C_PER_CORE, 128, 2048], f16,
                         kind="ExternalInput").ap()
    wyt = nc.dram_tensor("wyt", [C_PER_CORE, 128, 2048], f16,
                         kind="ExternalInput").ap()
    # out_dev[b, c, p, f*512 + a*256 + jo] = out[b, c*FN+f, a*128+p, jo]
    # (host transposes back; keeps store DMAs fully contiguous per partition)
    # fp16 output (|out| <~1e3, fp16 quantization ~5e-4 rel; host upcasts):
    # halves store traffic, and the kernel tail is store-drain bound.
    out = nc.dram_tensor("out", [B, C_PER_CORE, 128, 2048], f16,
                         kind="ExternalOutput").ap()

    with tile.TileContext(nc, pool_alloc_mode="queue") as tc:
        with (
            tc.tile_pool(name="wx", bufs=4) as wx_pool,
            tc.tile_pool(name="wy", bufs=4) as wy_pool,
            tc.tile_pool(name="xin", bufs=4) as x_pool,
            tc.tile_pool(name="warm", bufs=1) as warm_pool,
            tc.tile_pool(name="v", bufs=8) as v_pool,
            tc.tile_pool(name="osb", bufs=6) as o_pool,
            tc.tile_pool(name="psv", bufs=4, space=bass.MemorySpace.PSUM) as psv_pool,
            tc.tile_pool(name="pso", bufs=4, space=bass.MemorySpace.PSUM) as pso_pool,
        ):
            # Warm the PE clock gate (HAM) during the initial load
            # latency with dependency-free matmuls on scratch data.
            warm_sb = warm_pool.tile([128, 128], f16, tag="warm_sb",
                                     name="warm_sb")
            nc.vector.memset(warm_sb[:], 0.0)
            warm_ps = pso_pool.tile([128, 512], f32, tag="pso", name="pso")
            for _w in range(N_WARM):
                nc.tensor.matmul(warm_ps[:, :128], warm_sb[:], warm_sb[:],
                                 start=True, stop=True)

            # ---- prefetch all inputs up front ----
            xt_c = []
            wx_c = []
            wy_c = []
            for c in range(C_PER_CORE):
                xt_c.append(x_pool.tile([128, B * 512], f16, tag="x",
                                        name="x"))
                wx_c.append(wx_pool.tile([128, 2048], f16, tag="wx",
                                         name="wx"))
                wy_c.append(wy_pool.tile([128, 2048], f16, tag="wy",
                                         name="wy"))
            # Every SDMA engine round-robins across ACTIVE queues at packet
            # granularity, so each concurrent queue multiplies every
            # in-flight transfer's completion latency (measured: sem groups
            # spread 4-16us with 3 queues vs ~1us with 1).  The entire
            # critical load stream therefore rides ONE queue, strictly
            # need-ordered and fine-chunked so the first tiles consume
            # chunk-by-chunk right behind the delivery front; the PE warmup
            # bridges until the first chunks land.  c1-c3 follow as bulk
            # 512 KB whole-channel transfers on the same queue (done by
            # ~+30us; the SWDGE queue is empty long before the kernel tail
            # -> no end-of-kernel drain).
            nc.gpsimd.dma_start(xt_c[0][:, :256], xall[0][:, :256])
            nc.gpsimd.dma_start(xt_c[0][:, 256:512], xall[0][:, 256:512])
            for q in range(4):
                nc.gpsimd.dma_start(wx_c[0][:, q * 512:(q + 1) * 512],
                                    wxt[0][:, q * 512:(q + 1) * 512])
            nc.gpsimd.dma_start(wy_c[0][:, :1024], wyt[0][:, :1024])
            nc.gpsimd.dma_start(xt_c[0][:, 512:1024], xall[0][:, 512:1024])
            nc.gpsimd.dma_start(wy_c[0][:, 1024:], wyt[0][:, 1024:])
            nc.gpsimd.dma_start(xt_c[0][:, 1024:], xall[0][:, 1024:])
            for c in range(1, C_PER_CORE):
                nc.gpsimd.dma_start(wx_c[c][:], wxt[c])
                nc.gpsimd.dma_start(xt_c[c][:], xall[c])
                nc.gpsimd.dma_start(wy_c[c][:], wyt[c])

            for c in range(C_PER_CORE):
                wx_t = wx_c[c]
                wy_t = wy_c[c]
                for b in range(B):
                    xt = xt_c[c]
                    xb = b * 512

                    # Step 1: one PSUM bank per (jh, f-pair); per-half
                    # casts so step 2's f0/f1 can start after the fp0 copy.
                    vt = [v_pool.tile([128, 1024], f16, tag="v", name="v")
                          for _jh in range(2)]
                    for jh in range(2):
                        for fp in range(2):
                            psv = psv_pool.tile([128, 512], f32, tag="psv",
                                                name="psv")
                            for pc in range(2):
                                nc.tensor.matmul(
                                    psv[:],
                                    xt[:, xb + pc * 256 + jh * 128:
                                       xb + pc * 256 + jh * 128 + 128],
                                    wx_t[:, (fp * 2 + pc) * 512:
                                         (fp * 2 + pc) * 512 + 512],
                                    start=(pc == 0),
                                    stop=(pc == 1),
                                )
                            eng = (nc.vector.tensor_copy if (jh + fp) % 2 == 0
                                   else nc.scalar.copy)
                            eng(vt[jh][:, fp * 512:(fp + 1) * 512], psv[:])

                    # Step 2
                    osb = o_pool.tile([128, 2048], f16, tag="o", name="osb")
                    last = c == C_PER_CORE - 1 and b == B - 1
                    for f in range(FN):
                        pso = pso_pool.tile([128, 512], f32, tag="pso",
                                            name="pso")
                        for ih in range(2):
                            for jc in range(2):
                                nc.tensor.matmul(
                                    pso[:, ih * 256:(ih + 1) * 256],
                                    vt[jc][:, f * 256 + ih * 128:
                                           f * 256 + ih * 128 + 128],
                                    wy_t[:, (f * 2 + jc) * 256:
                                         (f * 2 + jc) * 256 + 256],
                                    start=(jc == 0),
                                    stop=(jc == 1),
                                )
                        dst = osb[:, f * 512:(f + 1) * 512]
                        if last and f == FN - 1:
                            # very last chunk: copy halves on both engines in
                            # parallel, then one paired [f2 f3] store on
                            # scalar (sync already carries [f0 f1]) -- two
                            # final store issues instead of four
                            nc.vector.tensor_copy(dst[:, :256], pso[:, :256])
                            nc.scalar.copy(dst[:, 256:], pso[:, 256:])
                            nc.scalar.dma_start(out[b, c][:, 1024:],
                                                osb[:, 1024:])
                        else:
                            eng = (nc.vector.tensor_copy if f % 2 == 0
                                   else nc.scalar.copy)
                            eng(dst[:], pso[:])
                            if last and f == 1:
                                # first half of the final tile stores as
                                # soon as f0/f1 are copied
                                nc.sync.dma_start(out[b, c][:, :1024],
                                                  osb[:, :1024])
                    if not last:
                        if c == C_PER_CORE - 1 and b == B - 2:
                            # split across both HWDGE queues near the end
                            nc.sync.dma_start(out[b, c][:, :1024],
                                              osb[:, :1024])
                            nc.scalar.dma_start(out[b, c][:, 1024:],
                                                osb[:, 1024:])
                        else:
                            # single contiguous 512 KB store (best DMA
                            # efficiency); alternate queues in c2 so the two
                            # HWDGE queues share the late store backlog, but
                            # keep scalar's queue clear during c3 (it is
                            # needed for tail copies + the final store)
                            eng = (nc.scalar if c == 2 and b % 2 == 1
                                   else nc.sync)
                            eng.dma_start(out[b, c], osb[:])

    nc.compile()
    return nc


def _get_program():
    if "nc" not in _PROGRAM_CACHE:
        _PROGRAM_CACHE["nc"] = _build_program()
    return _PROGRAM_CACHE["nc"]


def _band(mn, mx, dim):
    """Overlap weights W[i, p] of clipped window [i+mn, i+mx+1) with cell
    [p, p+1), built in fp64."""
    i = np.arange(dim, dtype=np.float64)[:, None]
    p = np.arange(dim, dtype=np.float64)[None, :]
    lo = i + float(mn)
    hi = i + float(mx) + 1.0
    return np.clip(p + 1.0 - lo, 0.0, 1.0) - np.clip(p + 1.0 - hi, 0.0, 1.0)


def _prepare_in_maps(input, x_min, x_max, y_min, y_max):
    # xall[c, p, b*512 + pc*256 + j] = x[b, c, pc*128 + p, j]
    # (b-major free dim per channel so each channel is one contiguous load)
    x16_full = np.ascontiguousarray(
        input.astype(np.float16).reshape(B, C, 2, 128, 256)
        .transpose(1, 3, 0, 2, 4).reshape(C, 128, B * 512))

    in_maps = []
    for core in range(N_CORES):
        c0 = core * C_PER_CORE
        wxt = np.empty((C_PER_CORE, 128, 2048), dtype=np.float16)
        wyt = np.empty((C_PER_CORE, 128, 2048), dtype=np.float16)
        for cl in range(C_PER_CORE):
            c = c0 + cl
            for f in range(FN):
                WxT = _band(x_min[c, f], x_max[c, f], H).T.astype(np.float16)
                WyT = _band(y_min[c, f], y_max[c, f], W).T.astype(np.float16)
                fp, fi = f // 2, f % 2
                for pc in range(2):
                    base = (fp * 2 + pc) * 512 + fi * 256
                    wxt[cl, :, base:base + 256] = WxT[pc * 128:(pc + 1) * 128]
                for jc in range(2):
                    base = (f * 2 + jc) * 256
                    wyt[cl, :, base:base + 256] = WyT[jc * 128:(jc + 1) * 128]
        in_maps.append({
            "xall": np.ascontiguousarray(x16_full[c0:c0 + C_PER_CORE]),
            "wxt": wxt,
            "wyt": wyt,
        })
    return in_maps


def run(input, x_min, x_max, y_min, y_max, trace=False):
    """Run the SPMD kernel; returns (full_output, BassKernelResults)."""
    from concourse.bass_utils import run_bass_kernel_spmd

    nc = _get_program()
    in_maps = _prepare_in_maps(
        np.asarray(input, dtype=np.float32),
        np.asarray(x_min, dtype=np.float64),
        np.asarray(x_max, dtype=np.float64),
        np.asarray(y_min, dtype=np.float64),
        np.asarray(y_max, dtype=np.float64),
    )
    res = run_bass_kernel_spmd(nc, in_maps, list(range(N_CORES)), trace=trace)
    # out_dev[b, c, p, f*512 + a*256 + jo] -> out[b, c*FN+f, a*128+p, jo]
    parts = []
    for i in range(N_CORES):
        o = res.results[i]["out"].astype(np.float32).reshape(
            B, C_PER_CORE, 128, FN, 2, 256)
        parts.append(o.transpose(0, 1, 3, 4, 2, 5).reshape(
            B, C_PER_CORE * FN, 256, 256))
    full = np.ascontiguousarray(np.concatenate(parts, axis=1))
    return full, res


def kernel(input, x_min, x_max, y_min, y_max):
    full, _ = run(input, x_min, x_max, y_min, y_max)
    return full
